# revision 1
# baseline (speedup 1.0000x reference)
"""Masked dot-product attention (B=16, LQ=LK=2048, D=64) on 8 TRN2 NeuronCores.

Strategy
--------
out[b] = softmax(mask(Q K^T / 8)) V, where keys >= valid_len[b] are masked.

Work decomposition: each (batch, q-half-of-1024) job costs
ceil(valid_len/128) k-tiles (keys beyond valid_len contribute exactly 0, so
they are never computed).  Jobs are split along k into segments of <= 8
k-tiles; segments are sorted by length and dealt 8-at-a-time into "slot
ranks" across the 8 cores.  The compiled loop length of rank r is the max
segment length within that rank, so every core runs the same instruction
stream while total work tracks the actual sparsity with little padding.

Per (slot, k-tile) on device, with keys on partitions ("S^T layout"):
  MM1   S^T[kk, q]   = (K^T chunk).T @ Q^T          (d=64 contraction, f32r)
  ACT   P[kk, q]     = exp(0.125 * S^T)
  MM2   acc[dd|1, q] += ([V chunk | ones]).T @ P    (k contraction, PSUM accum)
acc rows 0..63 are the unnormalized partial output^T over the segment's
k-range, row 64 the partial softmax denominator.  Masking: V rows and the
ones-column are zeroed (host-side) for keys >= valid_len or outside the
segment, so those keys contribute exactly 0 to numerator and denominator —
equivalent to the reference's -1e6 bias + softmax, with no mask work on
device.  Each slot DMAs its raw [65, 1024] accumulator out; the host sums
each job's segments and performs the final divide + transpose — both linear,
so host combining is exact.

Slots 2j and 2j+1 share "pair" input tensors (slot 2j on partitions 0-63,
slot 2j+1 on 64-127 for the q^T/k^T sections), split into a q+k DMA (gates
MM1) and a V DMA (gates MM2) so compute starts as soon as q+k lands.

float32r matmuls (single-pass fp32 mode, 1 cycle/row at N>=256 vs 4 for
plain fp32) cost ~2.7e-4 absmax relative error end-to-end.

Measured (cost-model timeline, seed-0 inputs): ~48 us/core — exp-activation
bound: ScalarE runs gapless through the main phase at 1 elem/cycle/lane,
with ~5 us input-DMA startup and ~5 us tail (final epilogue + Tile drain).
"""

import math
from contextlib import ExitStack

import numpy as np

import concourse.bacc as bacc
import concourse.mybir as mybir
import concourse.tile as tile
import concourse.bass_utils as bass_utils

B, LQ, LK, D = 16, 2048, 2048, 64
N_CORES = 8
KT = 128          # keys per k-tile (PSUM/partition granularity)
QS = 1024         # queries per slot (q-half)
SEG = 8           # max k-tiles per segment
SCALE = 1.0 / math.sqrt(D)

F32 = mybir.dt.float32
MM_DT = mybir.dt.float32r


def pair_layout(rank_lens, j):
    """Column offsets of the sections inside pair tensor j.

    Sections: qT (both slots stacked on partitions) | kT (same) |
    vp[slot 2j] | vp[slot 2j+1] (V chunk + ones column, zero for masked or
    out-of-segment keys).
    """
    na, nb = rank_lens[2 * j], rank_lens[2 * j + 1]
    qo = 0
    ko = qo + QS
    vo = [ko + na * KT, ko + na * KT + na * (D + 1)]
    width = vo[1] + nb * (D + 1)
    return qo, ko, vo, width


def build_bass(rank_lens, repeat=1, cfg=None):
    """Build the per-core Bass program for compiled slot lengths rank_lens.

    repeat>1 re-runs the whole computation N times inside the NEFF — used
    purely for device-time measurement by wall-clock differencing.
    """
    cf = {"sp": 3, "ap": 2, "pp": 4, "ep": 3}
    if cfg:
        cf.update(cfg)
    slots = len(rank_lens)
    pairs = slots // 2
    nc = bacc.Bacc("TRN2", target_bir_lowering=False, debug=False)

    widths = [pair_layout(rank_lens, j)[3] for j in range(pairs)]
    qk_w = [pair_layout(rank_lens, j)[2][0] for j in range(pairs)]
    pk = [
        nc.dram_tensor(f"pk{j}", [128, widths[j]], MM_DT, kind="ExternalInput").ap()
        for j in range(pairs)
    ]
    out = nc.dram_tensor("out", [slots * (D + 1), QS], F32, kind="ExternalOutput").ap()

    Exp = mybir.ActivationFunctionType.Exp

    with tile.TileContext(nc) as tc, ExitStack() as ctx:
        inp = ctx.enter_context(tc.tile_pool(name="inp", bufs=1))
        ppool = ctx.enter_context(tc.tile_pool(name="pp", bufs=cf["pp"]))
        epool = ctx.enter_context(tc.tile_pool(name="ep", bufs=cf["ep"]))
        spool = ctx.enter_context(tc.tile_pool(name="sp", bufs=cf["sp"], space="PSUM"))
        apool = ctx.enter_context(tc.tile_pool(name="ap", bufs=cf["ap"], space="PSUM"))

        # Shortest slots first: less input data has to land before compute
        # starts, and the heavy slots stream in behind.
        order = cf.get("order") or sorted(range(slots), key=lambda s: rank_lens[s])
        pair_order = sorted(range(pairs), key=lambda j: rank_lens[2 * j])

        qk_t = [None] * pairs
        km_t = [None] * pairs   # middle k-columns (3-way-split first pair)
        kx_t = [None] * pairs   # overflow k-columns beyond the shorter slot
        kx_at = [None] * pairs  # k-tile index where the overflow tile starts
        v_t = [None] * pairs
        j0 = pair_order[0]
        dma_seq = cf.get("dma_seq") or [
            (kind, j) for j in pair_order for kind in ("qk", "v")
        ]
        for kind, j in dma_seq:
            if kind == "qk":
                na, nb = rank_lens[2 * j], rank_lens[2 * j + 1]
                if j in pair_order[:2] and na > nb and nb > 1:
                    # 3-way split for the startup-critical first pair: the
                    # very first matmul waits only for q + one k-tile.
                    w1 = QS + KT
                    qk_t[j] = inp.tile([128, w1], MM_DT, name=f"qk{j}")
                    nc.sync.dma_start(qk_t[j][:], pk[j][:, :w1])
                    km_t[j] = inp.tile([128, (nb - 1) * KT], MM_DT, name=f"km{j}")
                    nc.sync.dma_start(km_t[j][:], pk[j][:, w1 : QS + nb * KT])
                    kx_t[j] = inp.tile([128, (na - nb) * KT], MM_DT, name=f"kx{j}")
                    kx_at[j] = nb
                    nc.sync.dma_start(kx_t[j][:], pk[j][:, QS + nb * KT : qk_w[j]])
                elif na > nb and (j == j0 or cf.get("split_all", True)):
                    # Split the startup-critical first-pair q+k transfer at
                    # the shorter slot's k-length: the first executed slot
                    # only waits for its own columns.
                    wa = QS + nb * KT
                    qk_t[j] = inp.tile([128, wa], MM_DT, name=f"qk{j}")
                    nc.sync.dma_start(qk_t[j][:], pk[j][:, :wa])
                    kx_t[j] = inp.tile([128, (na - nb) * KT], MM_DT, name=f"kx{j}")
                    kx_at[j] = nb
                    nc.sync.dma_start(kx_t[j][:], pk[j][:, wa : qk_w[j]])
                else:
                    qk_t[j] = inp.tile([128, qk_w[j]], MM_DT, name=f"qk{j}")
                    nc.sync.dma_start(qk_t[j][:], pk[j][:, : qk_w[j]])
            else:
                v_t[j] = inp.tile([128, widths[j] - qk_w[j]], MM_DT, name=f"v{j}")
                nc.sync.dma_start(v_t[j][:], pk[j][:, qk_w[j] :])

        for s in [s for _ in range(repeat) for s in order]:
            ns = rank_lens[s]
            j = s // 2          # pair index (shared input tile)
            pb = (s % 2) * 64   # partition base for q/k sections
            qo, ko, vo, _ = pair_layout(rank_lens, j)
            voff = vo[s % 2] - qk_w[j]
            pt = qk_t[j]
            accs = [
                apool.tile([D + 1, 512], F32, name=f"acc{s}_{qq}", tag="acc")
                for qq in range(2)
            ]

            for kt in range(ns):
                s_ps = spool.tile([128, QS], F32, name="s_ps")
                if kx_at[j] is not None and kt >= kx_at[j]:
                    kk = kt - kx_at[j]
                    lhsT = kx_t[j][pb : pb + 64, kk * KT : (kk + 1) * KT]
                elif km_t[j] is not None and kt >= 1:
                    lhsT = km_t[j][pb : pb + 64, (kt - 1) * KT : kt * KT]
                else:
                    lhsT = pt[pb : pb + 64, ko + kt * KT : ko + (kt + 1) * KT]
                for qq in range(2):
                    nc.tensor.matmul(
                        s_ps[:, qq * 512 : (qq + 1) * 512],
                        lhsT,
                        pt[pb : pb + 64, qo + qq * 512 : qo + (qq + 1) * 512],
                        start=True,
                        stop=True,
                    )
                p_t = ppool.tile([128, QS], MM_DT, name="p_t")
                nc.scalar.activation(p_t[:], s_ps[:], Exp, scale=SCALE)
                w = v_t[j][:, voff + kt * (D + 1) : voff + (kt + 1) * (D + 1)]
                for qq in range(2):
                    nc.tensor.matmul(
                        accs[qq][:, :],
                        w,
                        p_t[:, qq * 512 : (qq + 1) * 512],
                        start=(kt == 0),
                        stop=(kt == ns - 1),
                    )

            # Epilogue: raw partial accumulator straight to DRAM
            # (host does the segment-sum + divide + transpose).
            for qq in range(2):
                acc_sb = epool.tile([D + 1, 512], F32, name="acc_sb")
                nc.vector.tensor_copy(acc_sb[:], accs[qq][:])
                nc.sync.dma_start(
                    out[s * (D + 1) : (s + 1) * (D + 1), qq * 512 : (qq + 1) * 512],
                    acc_sb[:],
                )

    nc.compile()
    return nc


def plan_and_pack(queries, keys, values, valid_lens):
    """Split jobs into k-segments, deal into rank slots, gather inputs."""
    q = np.ascontiguousarray(np.asarray(queries, dtype=np.float32))
    k = np.asarray(keys, dtype=np.float32)
    v = np.asarray(values, dtype=np.float32)
    vl = np.asarray(valid_lens, dtype=np.int64)

    nkt = np.maximum(1, -(-vl // KT))  # ceil

    def make_segs(seg_max):
        segs = []  # (len, b, qh, k0)
        for b in range(B):
            n = int(nkt[b])
            m = -(-n // seg_max)
            base, rem = divmod(n, m)
            sizes = [base + 1] * rem + [base] * (m - rem)
            for qh in range(LQ // QS):
                k0 = 0
                for sz in sizes:
                    segs.append((sz, b, qh, k0))
                    k0 += sz
        segs.sort(key=lambda t: (-t[0], t[1], t[2], t[3]))
        return segs

    def cost(segs):
        ls = sorted((s[0] for s in segs), reverse=True)
        while len(ls) % N_CORES:
            ls.append(0)
        slots = len(ls) // N_CORES
        if slots % 2:
            slots += 1
            ls += [0] * N_CORES
        rsum = sum(max(ls[N_CORES * r], 1) for r in range(slots))
        return rsum * 1.072 + slots * 1.2  # us: ACT-paced unit + slot overhead

    seg_best = min(range(5, SEG + 1), key=lambda m: cost(make_segs(m)))
    segs = make_segs(seg_best)
    while len(segs) % N_CORES:
        segs.append(None)
    slots = len(segs) // N_CORES
    if slots % 2:  # pair structure needs an even slot count
        segs.extend([None] * N_CORES)
        slots += 1
    rank_lens = []
    for r in range(slots):
        first = segs[N_CORES * r]
        rank_lens.append(first[0] if first is not None else 1)
    pairs = slots // 2

    kT = np.swapaxes(k, 1, 2)  # [B, D, LK] view
    parts = np.arange(KT)

    in_maps = []
    slot_map = []  # per core: [(b, qh, k0) or None, ...] per slot
    for c in range(N_CORES):
        core_map = {}
        smap = []
        for j in range(pairs):
            qo, ko, vo, width = pair_layout(rank_lens, j)
            pkj = np.zeros((128, width), dtype=np.float32)
            for i, s in enumerate((2 * j, 2 * j + 1)):
                nr = rank_lens[s]  # compiled (padded) slot length
                seg = segs[N_CORES * s + c]
                if seg is None:
                    smap.append(None)
                    continue
                sz, b, qh, k0 = seg
                pb = i * 64
                smap.append((b, qh, k0))
                pkj[pb : pb + 64, qo : qo + QS] = q[b, qh * QS : (qh + 1) * QS, :].T
                kw = min(nr * KT, LK - k0 * KT)
                pkj[pb : pb + 64, ko : ko + kw] = kT[b, :, k0 * KT : k0 * KT + kw]
                vslab = pkj[:, vo[i] : vo[i] + nr * (D + 1)].reshape(128, nr, D + 1)
                nv = kw // KT
                vslab[:, :nv, :D] = (
                    v[b, k0 * KT : k0 * KT + nv * KT, :]
                    .reshape(nv, KT, D)
                    .transpose(1, 0, 2)
                )
                vslab[:, :, D] = 1.0
                # zero contributions of masked keys and keys outside the
                # segment's own range [k0, k0+sz)
                kid = (k0 + np.arange(nr))[None, :] * KT + parts[:, None]
                dead = (kid >= vl[b]) | (kid >= (k0 + sz) * KT)
                vslab[dead] = 0.0
            core_map[f"pk{j}"] = pkj
        in_maps.append(core_map)
        slot_map.append(smap)
    return rank_lens, in_maps, slot_map


def scatter_out(results, slot_map):
    acc = {}  # (b, qh) -> [65, QS] float64 partial sums
    for c in range(N_CORES):
        oc = results[c]["out"]
        for s, seg in enumerate(slot_map[c]):
            if seg is None:
                continue
            b, qh, _ = seg
            blk = oc[s * (D + 1) : (s + 1) * (D + 1), :].astype(np.float64)
            key = (b, qh)
            if key in acc:
                acc[key] += blk
            else:
                acc[key] = blk
    out = np.empty((B, LQ, D), dtype=np.float32)
    for (b, qh), a in acc.items():
        out[b, qh * QS : (qh + 1) * QS, :] = (a[:D, :] / a[D : D + 1, :]).T
    return out


def kernel(queries, keys, values, valid_lens, _run=None):
    rank_lens, in_maps, slot_map = plan_and_pack(queries, keys, values, valid_lens)
    nc = build_bass(rank_lens)
    if _run is not None:  # test hook (e.g. CoreSim)
        results = _run(nc, in_maps)
    else:
        import time as _time

        last = None
        for attempt in range(4):  # axon devices flake transiently under load
            try:
                results = bass_utils.run_bass_kernel_spmd(
                    nc, in_maps, core_ids=list(range(N_CORES))
                ).results
                break
            except Exception as e:  # noqa: BLE001
                last = e
                _time.sleep(45.0 * (attempt + 1))
        else:
            raise last
    return scatter_out(results, slot_map)

